# revision 7
# baseline (speedup 1.0000x reference)
"""Trainium2 Bass kernel for nn_PhongBase.

Math (per row n of inputs[N, 3, 3]):
    light  = inputs[n, 0, :]
    normal = inputs[n, 1, :]
    ndotl  = max(dot(light, normal), 0)
    out[n, j] = ks[j]/pi + kd[j]/pi * ndotl          (j = 0..2)

(The view vector inputs[n, 2, :] is unused; specular() == 1.0**alpha == 1.)

Strategy: pure data parallel over 8 NeuronCores. Each core gets N/8 rows as a
contiguous block, streams [128, W, 9] f32 tiles HBM->SBUF with fully
contiguous DMA, computes the dot product with strided DVE ops, applies the
per-channel affine (scale = kd/pi, bias = ks/pi, folded in as compile-time
immediates) on the scalar engine, and streams [128, W, 3] tiles back.
"""

import numpy as np

import concourse.bacc as bacc
import concourse.bass as bass
import concourse.mybir as mybir
from concourse.bass_utils import run_bass_kernel_spmd
from concourse.tile import TileContext

N_CORES = 8
N_ROWS = 8388608                  # full batch
P = 128                           # SBUF partitions
W = 1024                          # rows per partition per tile
T = N_ROWS // (N_CORES * P * W)   # tiles per core (= 8)

F32 = mybir.dt.float32
INV_PI = 1.0 / np.pi


def build_program(scale3, bias3, tiles=T, w=W):
    """Bass program for one core: x[tiles,P,w,9] -> y[tiles,P,w,3].

    out[..., j] = scale3[j] * relu(sum(x[..., 0:3] * x[..., 3:6], -1)) + bias3[j]
    """
    # Bacc (not plain Bass): its finalization passes split multi-wait
    # instructions via event semaphores — TRN2 allows 1 sync wait per inst.
    nc = bacc.Bacc(None)
    x = nc.dram_tensor("x", [tiles, P, w, 9], F32, kind="ExternalInput")
    y = nc.dram_tensor("y", [tiles, P, w, 3], F32, kind="ExternalOutput")

    with TileContext(nc) as tc:
        with (
            tc.tile_pool(name="const_pool", bufs=1) as const_pool,
            tc.tile_pool(name="in_pool", bufs=3) as in_pool,
            tc.tile_pool(name="out_pool", bufs=3) as out_pool,
            tc.tile_pool(name="dot_pool", bufs=3) as dot_pool,
        ):
            scale_t = const_pool.tile([P, 3], F32)
            bias_t = const_pool.tile([P, 3], F32)
            for j in range(3):
                nc.vector.memset(scale_t[:, j : j + 1], float(scale3[j]))
                nc.vector.memset(bias_t[:, j : j + 1], float(bias3[j]))
            scale_b = scale_t[:].unsqueeze(1).to_broadcast([P, w, 3])
            bias_b = bias_t[:].unsqueeze(1).to_broadcast([P, w, 3])

            for t in range(tiles):
                itile = in_pool.tile([P, w, 9], F32)
                otile = out_pool.tile([P, w, 3], F32)
                dot = dot_pool.tile([P, w], F32)

                nc.sync.dma_start(out=itile[:], in_=x[t])

                light = itile[:, :, 0:3]
                normal = itile[:, :, 3:6]
                # otile doubles as scratch for the elementwise products.
                nc.vector.tensor_mul(out=otile[:], in0=light, in1=normal)
                nc.vector.reduce_sum(
                    out=dot[:], in_=otile[:], axis=mybir.AxisListType.X
                )
                # relu in place; kept on DVE so every compute op shares one
                # engine (cross-engine deps cost scarce per-instruction sync
                # wait slots in this walrus pipeline)
                nc.vector.tensor_scalar_max(out=dot[:], in0=dot[:], scalar1=0.0)
                # out[:, :, j] = relu(dot) * scale[j] + bias[j], broadcast over
                # the channel axis; keeps a single last-writer for the store
                # DMA (walrus caps DMA sync-wait commands).
                dot_b = dot[:].unsqueeze(2).to_broadcast([P, w, 3])
                nc.vector.tensor_mul(out=otile[:], in0=dot_b, in1=scale_b)
                nc.vector.tensor_add(out=otile[:], in0=otile[:], in1=bias_b)

                nc.sync.dma_start(out=y[t], in_=otile[:])
    return nc


def run_sharded(x_np, scale3, bias3, **spmd_kwargs):
    """Shard x_np [N_ROWS, 3, 3] over 8 cores, run, gather [N_ROWS, 3]."""
    rows_per_core = N_ROWS // N_CORES
    x5 = np.ascontiguousarray(x_np, dtype=np.float32).reshape(
        N_CORES, T, P, W, 9
    )
    nc = build_program(scale3, bias3)
    nc.finalize()  # run Bacc's compile pipeline (wait splitting, reg alloc)
    in_maps = [{"x": x5[c]} for c in range(N_CORES)]
    rr = run_bass_kernel_spmd(nc, in_maps, list(range(N_CORES)), **spmd_kwargs)
    out = np.empty((N_CORES, rows_per_core, 3), dtype=np.float32)
    for c in range(N_CORES):
        out[c] = np.asarray(rr.results[c]["y"]).reshape(rows_per_core, 3)
    return out.reshape(N_ROWS, 3), rr


def kernel(inputs, kd, ks, alpha):
    inputs = np.asarray(inputs, dtype=np.float32)
    kd = np.asarray(kd, dtype=np.float32)
    ks = np.asarray(ks, dtype=np.float32)
    alpha = np.asarray(alpha, dtype=np.float32)

    inv_pi = np.float32(INV_PI)
    spec = np.float32(1.0) ** alpha          # specular() of the base class
    scale3 = (kd * inv_pi).astype(np.float32)          # per-channel scale
    bias3 = (ks * inv_pi * spec).astype(np.float32)    # per-channel bias

    out, _ = run_sharded(inputs, scale3, bias3)
    return out
